# revision 26
# baseline (speedup 1.0000x reference)
"""Trainium2 Bass kernel for a custom LSTM cell step.

Reference computation (per full problem, B=8192, D=U=512):
    z = inputs @ kernel + h_tm1 @ recurrent_kernel + bias        # [B, 4U]
    i, f, g, o = split(z, 4, axis=1)
    i, f, o = sigmoid(...)  ;  g = tanh(g)
    c = f * c_tm1 + i * g
    h = o * tanh(c)
    return (h, h, c)

Sharding: data-parallel over the batch dim across 8 NeuronCores
(1024 rows per core); kernel/recurrent_kernel/bias replicated.

Per-core kernel structure:
  - x and h are transposed on the HOST (xT/hT: [K, M] layout), so the PE
    does no transposes at all — 256 pure matmuls of [128k x 128m] @
    [128k x 512n], one gate (512 cols) per PSUM bank, all 8 banks used.
  - everything DMA'd is bf16 except nothing: x/h/W/R operands (1
    cycle/row on the PE, same rate as f32r but half the HBM traffic),
    c_tm1, and both outputs (host upcasts to f32). Measured rel_err
    3.4e-3 vs the 2e-2 gate; the old f32r path was WORSE (1.8e-2) —
    the PE's f32r in-array accumulation loses ~1e-2, bf16 + fp32 PSUM
    accumulation does not.
  - gate i is emitted k-major (ko octets across all 8 m-tiles) so the
    first matmul only needs xT[k0] + W_i[k0] to land; gates g/f/o are
    mt-major so each bank completes early and recycles through the ACT
    drain with no PE stall. The last m-tile's o-gate runs as 4 N=128
    chunks in separate banks so the final ACT/DVE/DMA chain is short.
  - N_WARM dummy matmuls keep the PE busy from ~7us (right after the
    framework preamble) so the ~5us p-state ramp to 2.4 GHz burns on
    throwaway work; they hand off seamlessly into the real stream at
    the moment the first weight/activation tiles land (~10.5us).
  - input DMAs are k-tile-granular and alternate between the sync and
    scalar rings (each ring only keeps a few DMAs in flight and each
    config op costs ~0.6us of sequencer time); the c chunks trail the
    weight stream since c is first needed in phase f. Outputs: c on the
    gpsimd ring, h alternating sync/gpsimd — never on the scalar ring,
    whose sequencer would stall ACT dispatch at the tail.
"""

from contextlib import ExitStack

import ml_dtypes
import numpy as np

import concourse.bass as bass
import concourse.mybir as mybir
import concourse.tile as tile
from concourse import bacc
from concourse.bass_utils import run_bass_kernel_spmd

# Problem sizes (hardcoded per spec).
B, D, U = 8192, 512, 512
N_CORES = 8
MB = B // N_CORES  # 1024 batch rows per core
P = 128
MT = MB // P  # 8 m-tiles per core
KO = (D + U) // P  # 8 stacked contraction tiles (4 from W/x, 4 from R/h)
NG = 4 * U  # 2048 gate columns

F32 = mybir.dt.float32
BF16 = mybir.dt.bfloat16

SIG = mybir.ActivationFunctionType.Sigmoid
TANH = mybir.ActivationFunctionType.Tanh

# Gate column order in the fused weight matrix: i, f, g, o.
COL = {"i": 0, "f": 1, "g": 2, "o": 3}

N_WARM = 15  # dummy N=256 matmuls to burn the PE p-state ramp during DMA wait

_NC_CACHE: dict = {}


def _build_lstm_nc(with_bias: bool):
    """Build and compile the per-core Bass program."""
    nc = bacc.Bacc("TRN2", target_bir_lowering=False, debug=False)

    xT_d = nc.dram_tensor("x_t", [D, MB], BF16, kind="ExternalInput")
    hT_d = nc.dram_tensor("h_t", [U, MB], BF16, kind="ExternalInput")
    c_d = nc.dram_tensor("c_tm1", [MB, U], BF16, kind="ExternalInput")
    w_d = nc.dram_tensor("kernel", [D, NG], BF16, kind="ExternalInput")
    r_d = nc.dram_tensor("recurrent_kernel", [U, NG], BF16, kind="ExternalInput")
    b_d = None
    if with_bias:
        b_d = nc.dram_tensor("bias", [NG], F32, kind="ExternalInput")
    # Outputs travel as bf16 (host upcasts to f32): halves output HBM
    # traffic and the tail DMA transfer.
    ho_d = nc.dram_tensor("h_out", [MB, U], BF16, kind="ExternalOutput")
    co_d = nc.dram_tensor("c_out", [MB, U], BF16, kind="ExternalOutput")

    # DRAM views tiled to [partition, tile, free]
    xT_v = xT_d.ap().rearrange("(ko p) m -> p ko m", p=P)  # [128, 4, 1024]
    hT_v = hT_d.ap().rearrange("(ko p) m -> p ko m", p=P)
    w_v = w_d.ap().rearrange("(ko p) n -> p ko n", p=P)  # [128, 4, 2048]
    r_v = r_d.ap().rearrange("(ko p) n -> p ko n", p=P)
    c_v = c_d.ap().rearrange("(mt p) d -> p mt d", p=P)
    ho_v = ho_d.ap().rearrange("(mt p) d -> p mt d", p=P)
    co_v = co_d.ap().rearrange("(mt p) d -> p mt d", p=P)

    with tile.TileContext(nc) as tc, ExitStack() as ctx:
        consts = ctx.enter_context(tc.tile_pool(name="consts", bufs=1))
        ipool = ctx.enter_context(tc.tile_pool(name="ipool", bufs=MT))
        igpool = ctx.enter_context(tc.tile_pool(name="igpool", bufs=MT))
        thpool = ctx.enter_context(tc.tile_pool(name="thpool", bufs=MT))
        scratch = ctx.enter_context(tc.tile_pool(name="scratch", bufs=3))
        outp = ctx.enter_context(tc.tile_pool(name="outp", bufs=4))
        zpsum = ctx.enter_context(tc.tile_pool(name="zpsum", bufs=8, space="PSUM"))

        # Stacked activations [k, m] and weights [k, n] in SBUF.
        xh = consts.tile([P, KO, MB], BF16)
        wr = consts.tile([P, KO, NG], BF16)
        c_sb = consts.tile([P, MT, U], BF16)

        # Warmup source: memset on the DVE, which is otherwise idle until
        # ~26us — it reaches this instruction right at its main-scope start
        # (~5.9us), about 1us before the gpsimd ring would (gpsimd spends
        # its early cycles on framework const memsets).
        warm = consts.tile([P, 384], BF16)
        nc.vector.memset(warm[:], 1.0)

        # --- input DMAs across three rings.
        # Each ring only holds a few DMAs in flight and each config op costs
        # ~0.6-1us of sequencer time, so: x + weights alternate on the
        # sync/scalar rings (k-tile paced for gate i), h k-tiles ride the
        # otherwise-idle gpsimd ring, and the c chunks trail the weight
        # stream (c is only needed from phase f, and a single big c DMA
        # earlier was found to block the ring for ~3.5us of descriptor-gen
        # and hog queue bandwidth exactly when the g/f weights were due).
        rings = [nc.sync, nc.scalar]
        ring_i = [0]

        def dma(dst, src):
            rings[ring_i[0] % len(rings)].dma_start(dst, src)
            ring_i[0] += 1

        ci = COL["i"] * U
        # x_k0 in halves: the first octet's mt0-3 matmuls unlock on
        # x_k0a + W_i_k0 (256KB) ~0.3us before the full k-tile would land.
        dma(xh[:, 0, 0:512], xT_v[:, 0, 0:512])
        dma(wr[:, 0, ci : ci + U], w_v[:, 0, ci : ci + U])
        dma(xh[:, 0, 512:MB], xT_v[:, 0, 512:MB])
        for ko in range(1, 4):  # x k-tiles + W_i k-tiles (gate i starts earliest)
            dma(xh[:, ko, :], xT_v[:, ko, :])
            dma(wr[:, ko, ci : ci + U], w_v[:, ko, ci : ci + U])
        for ko in range(4):  # h k-tiles + R_i k-tiles
            dma(xh[:, 4 + ko, :], hT_v[:, ko, :])
            dma(wr[:, 4 + ko, ci : ci + U], r_v[:, ko, ci : ci + U])

        def load_gate(name):
            cs = COL[name] * U
            dma(wr[:, 0:4, cs : cs + U], w_v[:, :, cs : cs + U])
            dma(wr[:, 4:8, cs : cs + U], r_v[:, :, cs : cs + U])

        load_gate("g")
        load_gate("f")
        dma(c_sb[:, 0:2, :], c_v[:, 0:2, :])
        dma(wr[:, 0:4, COL["o"] * U : (COL["o"] + 1) * U], w_v[:, :, COL["o"] * U : (COL["o"] + 1) * U])
        dma(c_sb[:, 2:4, :], c_v[:, 2:4, :])
        dma(wr[:, 4:8, COL["o"] * U : (COL["o"] + 1) * U], r_v[:, :, COL["o"] * U : (COL["o"] + 1) * U])
        dma(c_sb[:, 4:6, :], c_v[:, 4:6, :])
        dma(c_sb[:, 6:8, :], c_v[:, 6:8, :])

        bias_bc = None
        if with_bias:
            assert b_d is not None
            bias_bc = consts.tile([P, NG], F32)
            b_ap = b_d.ap()
            # DMA-replicate bias across all 128 partitions (partition step 0).
            nc.gpsimd.dma_start(
                out=bias_bc,
                in_=bass.AP(tensor=b_ap.tensor, offset=b_ap.offset, ap=[[0, P], [1, NG]]),
            )

        # --- PE warmup: dummy matmuls during the DMA wait window ---
        for wi in range(N_WARM):
            zw = zpsum.tile([P, U], F32, tag="z", name=f"zw{wi}")
            nc.tensor.matmul(
                zw[:, 0:256], warm[:, 0:P], warm[:, P : P + 256], start=True, stop=True
            )

        def mm(zp, ko, mt, gate):
            cs = COL[gate] * U
            nc.tensor.matmul(
                zp[:],
                xh[:, ko, mt * P : (mt + 1) * P],
                wr[:, ko, cs : cs + U],
                start=(ko == 0),
                stop=(ko == KO - 1),
            )

        def z_chunk(gate, mt):
            """Accumulate one gate's z columns for m-tile mt into a PSUM bank."""
            zp = zpsum.tile([P, U], F32, tag="z")
            for ko in range(KO):
                mm(zp, ko, mt, gate)
            return zp

        def add_bias(zp, gate):
            if bias_bc is not None:
                cs = COL[gate] * U
                nc.vector.tensor_add(zp[:], zp[:], bias_bc[:, cs : cs + U])

        i_t, ig_t, th_t = {}, {}, {}

        # Phase i: k-major octets — first octet only needs x_k0 + W_i_k0.
        zi = [zpsum.tile([P, U], F32, tag="z", name=f"zi{mt}") for mt in range(MT)]
        for ko in range(KO):
            for mt in range(MT):
                mm(zi[mt], ko, mt, "i")
        for mt in range(MT):
            add_bias(zi[mt], "i")
            it = ipool.tile([P, U], F32, tag="i")
            nc.scalar.activation(it[:], zi[mt][:], SIG)
            i_t[mt] = it

        # Phase g: g = tanh(z_g); ig = i*g
        for mt in range(MT):
            zp = z_chunk("g", mt)
            add_bias(zp, "g")
            gt = scratch.tile([P, U], F32, tag="gact")
            nc.scalar.activation(gt[:], zp[:], TANH)
            ig = igpool.tile([P, U], F32, tag="ig")
            nc.vector.tensor_mul(ig[:], i_t.pop(mt)[:], gt[:])
            ig_t[mt] = ig

        # Phase f: f = sigmoid(z_f); c = f*c_old + ig; tanh(c)
        for mt in range(MT):
            zp = z_chunk("f", mt)
            add_bias(zp, "f")
            ft = scratch.tile([P, U], BF16, tag="gact_bf")
            nc.scalar.activation(ft[:], zp[:], SIG)
            c_mul = scratch.tile([P, U], F32, tag="cmul")
            nc.vector.tensor_mul(c_mul[:], ft[:], c_sb[:, mt, :])
            c_new = outp.tile([P, U], BF16, tag="cnew")
            nc.vector.tensor_add(c_new[:], c_mul[:], ig_t.pop(mt)[:])
            nc.gpsimd.dma_start(co_v[:, mt, :], c_new[:])
            th = thpool.tile([P, U], F32, tag="th")
            nc.scalar.activation(th[:], c_new[:], TANH)
            th_t[mt] = th

        # Phase o: o = sigmoid(z_o); h = o*tanh(c)
        # The last m-tile is processed in two 256-col half-chunks so the
        # final ACT/DVE/DMA chain pipelines against the last matmuls.
        for mt in range(MT - 1):
            zp = z_chunk("o", mt)
            add_bias(zp, "o")
            ot = scratch.tile([P, U], F32, tag="gact")
            nc.scalar.activation(ot[:], zp[:], SIG)
            h_new = outp.tile([P, U], BF16, tag="hnew")
            nc.vector.tensor_mul(h_new[:], ot[:], th_t.pop(mt)[:])
            (nc.sync if mt % 2 == 0 else nc.gpsimd).dma_start(ho_v[:, mt, :], h_new[:])

        mt = MT - 1
        co = COL["o"] * U
        th_last = th_t.pop(mt)
        h_new = outp.tile([P, U], BF16, tag="hnew")
        NQ = 4  # quarter chunks: the final ACT/DVE/DMA chain is short
        for quart in range(NQ):
            qs = slice(quart * (U // NQ), (quart + 1) * (U // NQ))
            cols = slice(co + quart * (U // NQ), co + (quart + 1) * (U // NQ))
            # separate PSUM bank per chunk — avoids a WAR stall between one
            # chunk's matmuls and the previous chunk's sigmoid read
            zp = zpsum.tile([P, U], F32, tag="z", name=f"zo_q{quart}")
            for ko in range(KO):
                nc.tensor.matmul(
                    zp[:, 0 : U // NQ],
                    xh[:, ko, mt * P : (mt + 1) * P],
                    wr[:, ko, cols],
                    start=(ko == 0),
                    stop=(ko == KO - 1),
                )
            if bias_bc is not None:
                nc.vector.tensor_add(zp[:, 0 : U // NQ], zp[:, 0 : U // NQ], bias_bc[:, cols])
            ot = scratch.tile([P, U // NQ], F32, tag="gact_h", name=f"ot_q{quart}")
            nc.scalar.activation(ot[:], zp[:, 0 : U // NQ], SIG)
            nc.vector.tensor_mul(h_new[:, qs], ot[:], th_last[:, qs])
            (nc.gpsimd if quart % 2 == 0 else nc.sync).dma_start(ho_v[:, mt, qs], h_new[:, qs])

    nc.compile()
    return nc


def _get_nc(with_bias: bool):
    if with_bias not in _NC_CACHE:
        _NC_CACHE[with_bias] = _build_lstm_nc(with_bias)
    return _NC_CACHE[with_bias]


def _make_in_maps(inputs, h_tm1, c_tm1, kernel, recurrent_kernel, bias):
    """Shard full inputs into per-core input maps (host-side prep)."""
    x = np.asarray(inputs, dtype=np.float32).astype(ml_dtypes.bfloat16)
    h = np.asarray(h_tm1, dtype=np.float32).astype(ml_dtypes.bfloat16)
    c = np.asarray(c_tm1, dtype=np.float32).astype(ml_dtypes.bfloat16)
    w = np.ascontiguousarray(np.asarray(kernel, dtype=np.float32).astype(ml_dtypes.bfloat16))
    r = np.ascontiguousarray(
        np.asarray(recurrent_kernel, dtype=np.float32).astype(ml_dtypes.bfloat16)
    )
    b = np.ascontiguousarray(np.asarray(bias, dtype=np.float32))

    with_bias = bool(np.any(b))
    in_maps = []
    for core in range(N_CORES):
        sl = slice(core * MB, (core + 1) * MB)
        m = {
            "x_t": np.ascontiguousarray(x[sl].T),
            "h_t": np.ascontiguousarray(h[sl].T),
            "c_tm1": np.ascontiguousarray(c[sl]),
            "kernel": w,
            "recurrent_kernel": r,
        }
        if with_bias:
            m["bias"] = b
        in_maps.append(m)
    return in_maps, with_bias


def kernel(inputs, h_tm1, c_tm1, kernel, recurrent_kernel, bias):
    in_maps, with_bias = _make_in_maps(
        inputs, h_tm1, c_tm1, kernel, recurrent_kernel, bias
    )
    nc = _get_nc(with_bias)
    res = run_bass_kernel_spmd(nc, in_maps, core_ids=list(range(N_CORES)))
    h_out = np.concatenate(
        [np.asarray(r_["h_out"]).astype(np.float32) for r_ in res.results], axis=0
    )
    c_out = np.concatenate(
        [np.asarray(r_["c_out"]).astype(np.float32) for r_ in res.results], axis=0
    )
    return (h_out, h_out, c_out)
